# revision 18
# baseline (speedup 1.0000x reference)
"""LoRADense (per-token adapter routing) Bass kernel for 8 Trainium2 NeuronCores.

Math (reference):
    base  = x @ kernel + bias                      # (N, F)
    a     = lora_a[adapter_ids]                    # (N, D, R) gather
    b     = lora_b[adapter_ids]                    # (N, R, F) gather
    lr    = einsum('nd,ndr->nr', x, a)             # (N, R)
    delta = einsum('nr,nrf->nf', lr, b)            # (N, F)
    out   = base + delta

Strategy (v5):
  - GLOBAL sort of all 8192 tokens by adapter id on the host; core c gets the
    contiguous sorted run [1024c, 1024(c+1)).  Within a core, each 512-token
    chunk sees only ~5 consecutive adapter ids, so the host gathers, per
    (core, chunk), one 128-row band (8 adapters; spc slabs in general) of the
    concatenated LoRA factors, re-based so the device program is identical on
    every core (SPMD-safe).
  - Everything runs in bf16 (f32 PSUM accumulation), output stored bf16.
  - Transposed compute: out^T[f, tok] so the moving operand is always the
    token axis (512-wide chunks) and every stationary 128x128 block streams
    512 tokens:
      stage A: lr[sr_band, tok] = A_band^T @ x  (accumulate over 8 D-slabs),
               masked per (sr row, token) on DVE -> bf16 lrm in SBUF.
      stage B: po[f_blk, tok]   = sum_k Wk^T @ x  +  B_band^T @ lrm
               (one PSUM group of 8+spc matmuls), then +bias (per-partition
               scalar) fused with the f32->bf16 convert, DMA to DRAM.
  - k-major schedule in f-block passes sized to the 8 PSUM banks; pass 0
    carries stage A.  The per-k data (A band | x slab | first W f-blocks) is
    packed into ONE DMA per k so the stream feeds pass 0 just-in-time; the
    remaining W f-blocks stream during pass 1.
  - Host un-permutes rows and upcasts to f32.
"""

import numpy as np
import ml_dtypes

import concourse.bacc as bacc
import concourse.bass as bass
import concourse.mybir as mybir
import concourse.tile as tile
from concourse.bass_utils import run_bass_kernel_spmd

# Problem constants (hardcoded per harness contract).
N = 8192          # tokens
D = 1024          # input dim
F = 1024          # output features
R = 16            # lora rank
S = 64            # adapter slots
SR = S * R        # 1024
NCORES = 8
NTOK = N // NCORES            # 1024 tokens per core
P = 128                       # partitions
KD = D // P                   # 8 contraction slabs over D
TCH = 512                     # moving-operand token chunk
NCH = NTOK // TCH             # 2 chunks per core

BF16 = ml_dtypes.bfloat16

# Toggles (test.py pokes these).
TRACE = False
LAST_RESULTS = None
LAST_IN_MAPS = None
LAST_NC = None
LAST_NS = None

JUNK = 7
_NC_CACHE = {}


def _passes(spc):
    """f-block passes + whether stage A rides in pass 0, given PSUM budget 8."""
    n_lr = NCH * spc
    if n_lr <= 8 - NCH:  # room for at least one f-block next to the lr banks
        g0 = (8 - n_lr) // NCH
        jgs = [tuple(range(g0))]
        a_in_pass0 = True
    else:
        jgs = []
        a_in_pass0 = False
        g0 = 0
    j = g0
    while j < KD:
        g = min(8 // NCH, KD - 1 - j) if j < KD - 1 else 1
        g = max(1, min(g, KD - j - 1 if KD - j > 1 else 1))
        jgs.append(tuple(range(j, j + g)))
        j += g
    return jgs, a_in_pass0


def _build_nc(spc):
    """Build the single-core Bass program (same program runs on all 8 cores).

    spc = LoRA slabs (128-row bands) per 512-token chunk; normally 1.
    """
    f32 = mybir.dt.float32
    bf16 = mybir.dt.bfloat16
    nsl = NCH * spc                 # total gathered slabs per core
    jgs, a_in_p0 = _passes(spc)
    nja = len(jgs[0]) if a_in_p0 else 0   # f-blocks packed with the k-stream
    ACW = nsl * P                   # A-band columns in the pack
    XO = ACW                        # x offset in the pack
    WO = ACW + NTOK                 # W offset in the pack
    PKW = WO + nja * P              # pack width (bf16 elements)
    NJB = KD - nja                  # f-blocks in the second W stream

    nc = bacc.Bacc("TRN2", target_bir_lowering=False, debug=False)

    # DRAM I/O. Layouts are pre-packed on the host so every DMA is a plain
    # contiguous [partition, free...] copy.
    pk = nc.dram_tensor("pk", [P, KD, PKW], bf16, kind="ExternalInput")
    wkb = nc.dram_tensor("wkb", [P, KD, NJB * P], bf16, kind="ExternalInput")
    bs = nc.dram_tensor("bs", [P, nsl, F], bf16, kind="ExternalInput")
    msk = nc.dram_tensor("msk", [P, spc, NTOK], bf16, kind="ExternalInput")
    bia = nc.dram_tensor("bia", [P, KD], f32, kind="ExternalInput")
    out_s = nc.dram_tensor("out_s", [KD, P, NTOK], bf16, kind="ExternalOutput")

    with tile.TileContext(nc) as tc:
        with (
            tc.tile_pool(name="const", bufs=1) as cpool,
            tc.tile_pool(name="work", bufs=4) as wpool,
            tc.tile_pool(name="accp", bufs=8, space="PSUM") as accp,
        ):
            # Just-in-time DMA stream: one pack per D-slab k feeds pass 0.
            pk_sb = cpool.tile([P, KD, PKW], bf16)
            nc.sync.dma_start(pk_sb[:, 0, :XO + TCH], pk[:, 0, :XO + TCH])
            nc.sync.dma_start(pk_sb[:, 0, XO + TCH:], pk[:, 0, XO + TCH:])
            for k in range(1, KD):
                nc.sync.dma_start(pk_sb[:, k], pk[:, k])
            msk_sb = cpool.tile([P, spc, NTOK], bf16)
            nc.sync.dma_start(msk_sb[:], msk[:])
            bia_sb = cpool.tile([P, KD], f32)
            nc.sync.dma_start(bia_sb[:], bia[:])
            bs_sb = cpool.tile([P, nsl, F], bf16)
            nc.sync.dma_start(bs_sb[:], bs[:])
            wkb_sb = cpool.tile([P, KD, NJB * P], bf16)
            for k in range(KD):
                nc.sync.dma_start(wkb_sb[:, k], wkb[:, k])

            def wblk(k, j):
                if j < nja:
                    return pk_sb[:, k, WO + j * P:WO + (j + 1) * P]
                return wkb_sb[:, k, (j - nja) * P:(j - nja + 1) * P]

            # Masked low-rank activations, bf16: [sr_p, chunk-band, tok]
            lrm_sb = cpool.tile([P, spc, NTOK], bf16)

            # Warm-up: keep the PE busy (and the HAM clock-gate ramping)
            # while the first input packs are still in flight.  The junk
            # accumulator borrows one accp slot and is released before the
            # last pass-0 group needs its bank.
            junk_sb = cpool.tile([P, P], bf16)
            nc.vector.memset(junk_sb[:], 0.0)
            jp = accp.tile([P, TCH], mybir.dt.float32, tag="acc", name="jp")
            for w in range(JUNK):
                nc.tensor.matmul(
                    jp[:, :P], junk_sb[:], junk_sb[:],
                    start=True, stop=True,
                )

            def stage_a(t, o, k, ps):
                tok = slice(t * TCH, (t + 1) * TCH)
                nc.tensor.matmul(
                    ps[:],
                    pk_sb[:, k, (t * spc + o) * P:(t * spc + o + 1) * P],
                    pk_sb[:, k, XO + t * TCH:XO + (t + 1) * TCH],
                    start=(k == 0),
                    stop=(k == KD - 1),
                )
                if k == KD - 1:
                    # msk[p, o, tok] = (lid[tok] == (o*128+p)//16), host-built
                    nc.vector.tensor_tensor(
                        lrm_sb[:, o, tok],
                        ps[:],
                        msk_sb[:, o, tok],
                        mybir.AluOpType.mult,
                    )

            obs = {}

            def close_group(t, j, po):
                tok = slice(t * TCH, (t + 1) * TCH)
                for o in range(spc):
                    nc.tensor.matmul(
                        po[:],
                        bs_sb[:, t * spc + o, j * P:(j + 1) * P],
                        lrm_sb[:, o, tok],
                        start=False,
                        stop=(o == spc - 1),
                    )
                if j not in obs:
                    obs[j] = wpool.tile([P, NTOK], bf16, tag="ob",
                                        name=f"ob_{j}")
                nc.any.tensor_scalar_add(obs[j][:, tok], po[:],
                                         bia_sb[:, j:j + 1])
                if j == KD - 1:
                    # last f-block: per-chunk DMA so the first half overlaps
                    # the final chunk's close + convert
                    nc.sync.dma_start(out_s[j, :, tok], obs[j][:, tok])
                elif t == NCH - 1:
                    nc.sync.dma_start(out_s[j], obs[j][:])

            run_a = a_in_p0
            if not a_in_p0:
                # Fallback: sequential stage A before the f-block passes.
                for t in range(NCH):
                    for o in range(spc):
                        ps = accp.tile([P, TCH], mybir.dt.float32, tag="acc",
                                       name=f"lr_{t}_{o}")
                        for k in range(KD):
                            stage_a(t, o, k, ps)

            for gi, jg in enumerate(jgs):
                last = gi == len(jgs) - 1
                pos = {}
                lrs = {}
                for t in range(NCH):
                    for j in jg:
                        pos[(t, j)] = accp.tile(
                            [P, TCH], mybir.dt.float32, tag="acc",
                            name=f"po_{t}_{j}")
                    if gi == 0 and run_a:
                        for o in range(spc):
                            lrs[(t, o)] = accp.tile(
                                [P, TCH], mybir.dt.float32, tag="acc",
                                name=f"lr_{t}_{o}")
                if last:
                    # t-major: the first chunk's close/convert/DMA overlaps
                    # the second chunk's matmuls, shortening the tail.
                    for t in range(NCH):
                        for k in range(KD):
                            for j in jg:
                                nc.tensor.matmul(
                                    pos[(t, j)][:],
                                    wblk(k, j),
                                    pk_sb[:, k,
                                          XO + t * TCH:XO + (t + 1) * TCH],
                                    start=(k == 0),
                                    stop=False,
                                )
                        for j in jg:
                            close_group(t, j, pos[(t, j)])
                    continue
                for k in range(KD):
                    for t in range(NCH):
                        if gi == 0 and run_a:
                            for o in range(spc):
                                stage_a(t, o, k, lrs[(t, o)])
                        for j in jg:
                            nc.tensor.matmul(
                                pos[(t, j)][:],
                                wblk(k, j),
                                pk_sb[:, k, XO + t * TCH:XO + (t + 1) * TCH],
                                start=(k == 0),
                                stop=False,
                            )
                for t in range(NCH):
                    for j in jg:
                        close_group(t, j, pos[(t, j)])

    nc.compile()
    return nc


def _get_nc(spc):
    key = (spc, JUNK)
    if key not in _NC_CACHE:
        _NC_CACHE[key] = _build_nc(spc)
    return _NC_CACHE[key]


def kernel(x, adapter_ids, kernel, bias, lora_a, lora_b):
    global LAST_RESULTS, LAST_IN_MAPS, LAST_NC, LAST_NS
    x = np.ascontiguousarray(np.asarray(x, dtype=np.float32))
    adapter_ids = np.asarray(adapter_ids)
    kernel_w = np.asarray(kernel, dtype=np.float32)
    bias = np.asarray(bias, dtype=np.float32)
    lora_a = np.asarray(lora_a, dtype=np.float32)
    lora_b = np.asarray(lora_b, dtype=np.float32)
    ids = adapter_ids.astype(np.int64)

    # Global stable sort by adapter id; each core gets a contiguous run.
    perm = np.argsort(ids, kind="stable")
    ids_s = ids[perm]
    xs_all = x[perm]

    # Per-(core, chunk) adapter band [a0, a0 + 8*spc).
    spans = []
    for cc in range(NCORES * NCH):
        blk = ids_s[cc * TCH:(cc + 1) * TCH]
        spans.append(int(blk.max()) - int(blk.min()) + 1)
    spc = max(1, int(np.ceil(max(spans) / 8)))
    a0s = []
    for cc in range(NCORES * NCH):
        blk = ids_s[cc * TCH:(cc + 1) * TCH]
        a0s.append(min(int(blk.min()), S - 8 * spc) if 8 * spc < S else 0)

    nsl = NCH * spc
    jgs, a_in_p0 = _passes(spc)
    nja = len(jgs[0]) if a_in_p0 else 0
    ACW = nsl * P
    XO = ACW
    WO = ACW + NTOK
    PKW = WO + nja * P
    NJB = KD - nja

    # Replicated weight layouts with contiguous per-partition runs.
    a_cat = lora_a.transpose(1, 0, 2).reshape(D, SR)                  # (D, S*R)
    b_stk = lora_b.reshape(SR, F)                                     # (S*R, F)
    # wk4[k, p, j, fi] = kernel[k*128+p, j*128+fi]
    wk4 = kernel_w.reshape(KD, P, KD, P).astype(BF16)
    wkb_l = np.ascontiguousarray(
        wk4[:, :, nja:, :].reshape(KD, P, NJB * P).transpose(1, 0, 2))
    bia_l = np.ascontiguousarray(bias.reshape(KD, P).T.astype(np.float32))

    # Per-(slab-row, band-slab) local adapter index: (o*128+p)//16
    adiv = (np.arange(spc)[None, :] * P + np.arange(P)[:, None]) // R  # (P, spc)

    in_maps = []
    for c in range(NCORES):
        lo = c * NTOK
        xs = xs_all[lo:lo + NTOK]                                     # (NTOK, D)
        ac_g = np.empty((D, nsl * P), dtype=BF16)
        bs_g = np.empty((nsl, P, F), dtype=BF16)
        msk_l = np.empty((P, spc, NTOK), dtype=BF16)
        for t in range(NCH):
            a0 = a0s[c * NCH + t]
            sr0 = a0 * R
            ac_g[:, (t * spc) * P:(t * spc + spc) * P] = \
                a_cat[:, sr0:sr0 + spc * P].astype(BF16)
            bs_g[t * spc:(t + 1) * spc] = \
                b_stk[sr0:sr0 + spc * P].reshape(spc, P, F).astype(BF16)
            lid = ids_s[lo + t * TCH: lo + (t + 1) * TCH] - a0        # (TCH,)
            msk_l[:, :, t * TCH:(t + 1) * TCH] = \
                (adiv[:, :, None] == lid[None, None, :]).astype(BF16)
        # Pack [A band | x^T | first W f-blocks] per D-slab k.
        pk_l = np.empty((P, KD, PKW), dtype=BF16)
        pk_l[:, :, :ACW] = ac_g.reshape(KD, P, ACW).transpose(1, 0, 2)
        pk_l[:, :, XO:WO] = \
            xs.T.reshape(KD, P, NTOK).transpose(1, 0, 2).astype(BF16)
        pk_l[:, :, WO:] = \
            wk4[:, :, :nja, :].reshape(KD, P, nja * P).transpose(1, 0, 2)
        bs_l = np.ascontiguousarray(bs_g.transpose(1, 0, 2))
        in_maps.append({
            "pk": np.ascontiguousarray(pk_l), "wkb": wkb_l, "bs": bs_l,
            "msk": np.ascontiguousarray(msk_l), "bia": bia_l,
        })

    nc = _get_nc(spc)
    res = run_bass_kernel_spmd(nc, in_maps, core_ids=list(range(NCORES)),
                               trace=TRACE)
    LAST_RESULTS = res
    LAST_IN_MAPS = in_maps
    LAST_NC = nc
    LAST_NS = spc

    out = np.empty((N, F), dtype=np.float32)
    for c in range(NCORES):
        # out_s[j, p, t] holds out^T for f = j*128+p -> reshape to (F, NTOK).
        core_out = res.results[c]["out_s"].reshape(F, NTOK).T
        out[perm[c * NTOK:(c + 1) * NTOK]] = core_out.astype(np.float32)
    return out
